# revision 6
# baseline (speedup 1.0000x reference)
"""RotatE KGE scoring kernel for Trainium2 (Bass/Tile), 8-core data parallel. v3.

Problem (per reference):
  head  = entity_embedding[head_part[:,0]]           # [B,1,1000]
  rel   = relation_embedding[head_part[:,1]]         # [B,1,500]
  tail  = entity_embedding[tail_part]                # [B,128,1000]
  phase = rel / (EMB_RANGE/PI); rot = head * e^{i*phase}  (complex, D/2=500)
  score = GAMMA - sum_d sqrt((rot_re-tail_re)^2 + (rot_im-tail_im)^2)

Sharding: batch dim (1024) split across 8 cores, 128 batches each; embedding
tables replicated. Dominant cost per core: gathering 128x128 entity rows x
4KB = 65.5 MB from HBM (memory-bound).

v2 changes vs v1 (which ran every engine at 75-90% and was latency-limited):
  - Host interleaves entity columns (re_d, im_d adjacent) so a single custom
    DVE op computes sq_re+sq_im pair sums in one 1-elem/cycle stream:
    2-state FSM (reset pair accumulator / combine+write). This removes BOTH
    identity matmuls and the separate pair-add: the PE is now fully idle,
    and DVE drops from 1.62us/j to ~1.2us/j.
  - Host sorts each batch row's neg indices (output unpermuted on host):
    each gather's 128 rows then cluster in a narrow band of the entity table
    (order statistics), improving HBM row locality.
  - Gathers stay at 1 row/partition per indirect DMA: HW SWDGE applies ONE
    dynamic offset per partition (a [128,k] offset AP reads k*D contiguous
    elements from row idx[p,0] on, unlike the bass simulator).
  - The first 12 gathers are emitted ahead of the head/rel gathers and the
    trig chain in POOL program order, so the SDMA stream starts during the
    preamble; 20 tail buffers keep the descriptor ring fed.

Measured pacing (238,963ns total): POOL executes each DMA_INDIRECT in
~1.36us plus a fixed ~325ns sequencer gap -> ~215us for 128 gathers; DVE
pairsum ~1.2us/j and ACT sqrt+accum ~0.96us/j ride under it; PE unused.
"""

import math
from contextlib import ExitStack

import numpy as np

import concourse.bacc as bacc
import concourse.mybir as mybir
import concourse.tile as tile
from concourse.bass import IndirectOffsetOnAxis
from concourse.bass_utils import run_bass_kernel_spmd

# ---- problem constants (hardcoded per contract) ----
N_CORES = 8
B = 1024
B_LOC = B // N_CORES  # 128
NEG = 128
N_ENT = 100000
N_REL = 500
D = 1000
D2 = D // 2  # 500
G = 1  # j's processed per DVE pairsum op
NSTEP = NEG // G  # 128

GAMMA = 12.0
EPSILON = 2.0
EMB_RANGE = (GAMMA + EPSILON) / D2  # 0.028
PI = 3.141592653589793
PHASE_SCALE = float(1.0 / (EMB_RANGE / PI))  # multiply instead of divide

TWO_PI = 2.0 * math.pi
INV_TWO_PI = 1.0 / TWO_PI
MAGIC = 1.5 * 2.0**23  # round-to-nearest via fp32 quantization
# Cody-Waite split of 2*pi: c0 exact in fp32, c1 fp32, c2 the f64 remainder
CW0 = 6.28125
CW1 = float(np.float32(TWO_PI - CW0))
CW2 = float(TWO_PI - CW0 - np.float64(np.float32(TWO_PI - CW0)))

f32 = mybir.dt.float32
i32 = mybir.dt.int32
AF = mybir.ActivationFunctionType

_CACHED_NC = None
_PAIRSUM_OP = None


PAIRSUM_VARIANT = "G2"  # "G2": compact out [P,N/2]; "F": full out (sums at odd k)


def _register_pairsum():
    """Custom DVE op: pairwise sum of squared differences.

    G2 (compact): out[p,s] = (in0-in1)^2[p,2s] + (in0-in1)^2[p,2s+1], [P,N/2].
    F  (full):    out[p,k] = running pair sum (resets every 2), sums at odd k.

    The Spec DSL's scan cannot express a per-page reset, so the FSM is
    hand-assembled from lower()'s internals: seed bubble (uop index 0 is
    IDLE in next_uop references, so no state may be re-entered at 0) ->
    reset (BYPASS(sq) override on the scan combine stage, one element) ->
    combine (ADD(CURR, sq), one element, writes) -> back to reset.
    The compiled uops are seeded into dve_ops._COMPILE_CACHE so table-gen
    and trace-time compile() use them (the declarative lower() path would
    produce a plain cumulative scan).
    """
    global _PAIRSUM_OP
    if _PAIRSUM_OP is not None:
        return _PAIRSUM_OP
    import concourse.dve_ops as dve_ops
    from concourse.dve_spec import (
        Spec, Src0, Src1, sq, scan, AluOp, _collect, _validate_body,
        _hoist_stream_invariant_ops, _build_placement, _assemble, _State,
        _Stage, Scan, _scan_overrides,
    )
    from concourse.dve_uop import DveOpSpec, N_LANES, N_STAGES, Trigger

    name = f"SQD_SCAN_{PAIRSUM_VARIANT}"
    if name in dve_ops._SUB_OPCODE_FOR_NAME:
        _PAIRSUM_OP = next(op for op in dve_ops.OPS if op.name == name)
        return _PAIRSUM_OP

    body_expr = sq(Src0 - Src1)
    scan_node = scan(AluOp.ADD, body_expr)
    spec = Spec(
        body=scan_node,
        reference=lambda in0, in1, s0, s1, imm2: np.cumsum(
            (in0 - in1).astype(np.float32) ** 2, axis=-1
        ),
    )
    opcode = dve_ops._CUSTOM_DVE_ROW_BASE + len(dve_ops.OPS)
    assert opcode < 0x20

    shas = {}
    compiled = {}
    for ver in ("v3", "v4"):
        n_lanes, n_stages = N_LANES[ver], N_STAGES[ver]
        _validate_body(spec, ver)
        spec2 = _hoist_stream_invariant_ops(spec)
        scans = _collect(spec2.body, Scan)
        placement = _build_placement(spec2, scans, n_stages, n_lanes)
        scan_stage = placement.node_stage[scans[0]]
        reset_ov = {scan_stage: _Stage(AluOp.BYPASS, scans[0].expr)}
        seed_ov, _ = _scan_overrides(scans, placement.node_stage)
        st_seed = _State(
            placement=placement, overrides=seed_ov,
            trigger=(Trigger.COUNT, Trigger.NONE, Trigger.NONE),
            next=(1, 0, 0), repeat=1, write_out=False,
        )
        st_reset = _State(
            placement=placement, consume=(True, True), overrides=reset_ov,
            write_out=(PAIRSUM_VARIANT == "F"),
            trigger=(Trigger.SRC_TENSOR_DONE, Trigger.COUNT, Trigger.NONE),
            next=(0, 2, 0), repeat=1,
        )
        st_comb = _State(
            placement=placement, consume=(True, True),
            trigger=(Trigger.SRC_TENSOR_DONE, Trigger.COUNT, Trigger.NONE),
            next=(0, 1, 0), repeat=1,
        )
        uops = [_assemble(s) for s in (st_seed, st_reset, st_comb)]
        for u in uops:
            u.validate(ver)
        ds = DveOpSpec(name=name, opcode=opcode, uops=uops, rd1_en=True)
        shas[ver] = ds.sha(ver)
        compiled[ver] = ds
    op = dve_ops.DveOp(name, spec, subdim=False, uops_sha=shas)
    dve_ops.OPS.append(op)
    dve_ops._SUB_OPCODE_FOR_NAME[name] = opcode
    dve_ops.CUSTOM_DVE_SPECS[name] = spec
    for ver in ("v3", "v4"):
        dve_ops._COMPILE_CACHE[(name, ver)] = compiled[ver]
    _PAIRSUM_OP = op
    return op


def _build_nc():
    pairsum = _register_pairsum()
    nc = bacc.Bacc("TRN2", target_bir_lowering=False, debug=False)

    hp = nc.dram_tensor("head_part", [B_LOC, 3], i32, kind="ExternalInput")
    tp = nc.dram_tensor("tail_part", [B_LOC, NEG], i32, kind="ExternalInput")
    rel = nc.dram_tensor("relation_embedding", [N_REL, D2], f32, kind="ExternalInput")
    # entity table is column-INTERLEAVED on the host: ent_il[:, 2d]=re_d, [:, 2d+1]=im_d
    ent = nc.dram_tensor("entity_embedding", [N_ENT, D], f32, kind="ExternalInput")
    score = nc.dram_tensor("score", [B_LOC, NEG], f32, kind="ExternalOutput")

    P = 128

    with tile.TileContext(nc) as tc, ExitStack() as ctx:
        const = ctx.enter_context(tc.tile_pool(name="const", bufs=1))
        pre = ctx.enter_context(tc.tile_pool(name="pre", bufs=1))
        tails = ctx.enter_context(tc.tile_pool(name="tails", bufs=26))
        sqp = ctx.enter_context(tc.tile_pool(name="sqp", bufs=6))
        psc = ctx.enter_context(tc.tile_pool(name="psc", bufs=2, space="PSUM"))

        # ---------- preamble ----------
        tp_t = const.tile([P, NEG], i32)
        nc.sync.dma_start(out=tp_t[:], in_=tp[:])
        hp_t = const.tile([P, 3], i32)
        nc.sync.dma_start(out=hp_t[:], in_=hp[:])

        def emit_gather(s):
            tj = tails.tile([P, G * D], f32, tag="tj", name=f"tj{s}")
            nc.gpsimd.indirect_dma_start(
                out=tj[:], out_offset=None, in_=ent[:],
                in_offset=IndirectOffsetOnAxis(ap=tp_t[:, s : s + 1], axis=0),
            )
            return tj

        # hoist the first gathers ahead of the head/rel gathers and trig chain
        # in POOL program order: they only depend on tp_t, and the SDMA stream
        # starts ~6us earlier. HOIST < tails bufs so no buffer-reuse wait can
        # deadlock against rot2 (computed below).
        HOIST = 24
        hoisted = [emit_gather(s) for s in range(HOIST)]

        head_t = pre.tile([P, D], f32)  # interleaved (re_d, im_d)
        nc.gpsimd.indirect_dma_start(
            out=head_t[:], out_offset=None, in_=ent[:],
            in_offset=IndirectOffsetOnAxis(ap=hp_t[:, 0:1], axis=0),
        )
        relv = pre.tile([P, D2], f32)
        nc.gpsimd.indirect_dma_start(
            out=relv[:], out_offset=None, in_=rel[:],
            in_offset=IndirectOffsetOnAxis(ap=hp_t[:, 1:2], axis=0),
        )

        def const_col(val):
            t = const.tile([P, 1], f32, tag=f"c{val}")
            nc.gpsimd.memset(t[:], float(val))
            return t[:]

        b_magic = const_col(MAGIC)
        b_negmagic = const_col(-MAGIC)
        b_halfpi = const_col(PI / 2.0)
        b_gamma = const_col(GAMMA)

        # phase = relv * PHASE_SCALE; range-reduce to [-pi, pi]
        phase = pre.tile([P, D2], f32)
        nc.scalar.activation(phase[:], relv[:], AF.Identity, scale=PHASE_SCALE)
        t1 = pre.tile([P, D2], f32)
        nc.scalar.activation(t1[:], phase[:], AF.Identity, scale=INV_TWO_PI, bias=b_magic)
        kf = pre.tile([P, D2], f32)
        nc.scalar.activation(kf[:], t1[:], AF.Identity, bias=b_negmagic)
        ws = pre.tile([P, D2], f32)
        nc.vector.cody_waite_cascade(ws[:], phase[:], kf[:], CW0, CW1, CW2)

        # im_rel = sin(ws); re_rel = cos(ws) = sin(pi/2 - |ws|)
        im_rel = pre.tile([P, D2], f32)
        nc.scalar.activation(im_rel[:], ws[:], AF.Sin)
        aws = pre.tile([P, D2], f32)
        nc.scalar.activation(aws[:], ws[:], AF.Abs)
        re_rel = pre.tile([P, D2], f32)
        nc.scalar.activation(re_rel[:], aws[:], AF.Sin, scale=-1.0, bias=b_halfpi)

        # rot (interleaved): rot[2d] = he_d*cos_d - hi_d*sin_d
        #                    rot[2d+1] = he_d*sin_d + hi_d*cos_d
        # where he = head[2d], hi = head[2d+1] (strided views).
        # All ops are tensor_tensor class (never contend with SWDGE).
        he = head_t[:, 0:D:2]
        hi = head_t[:, 1:D:2]
        rot2 = pre.tile([P, G * D], f32)
        m_re = pre.tile([P, D2], f32)
        nc.vector.tensor_mul(m_re[:], he, re_rel[:])
        m_im = pre.tile([P, D2], f32)
        nc.vector.tensor_mul(m_im[:], hi, im_rel[:])
        nc.vector.tensor_sub(rot2[:, 0:D:2], m_re[:], m_im[:])
        m2 = pre.tile([P, D2], f32)
        nc.vector.tensor_mul(m2[:], he, im_rel[:])
        m3 = pre.tile([P, D2], f32)
        nc.vector.tensor_mul(m3[:], hi, re_rel[:])
        nc.vector.tensor_add(rot2[:, 1:D:2], m2[:], m3[:])
        # replicate rot into the remaining G-1 slots (ACT copies; preamble-only)
        for i in range(1, G):
            nc.scalar.activation(rot2[:, i * D : (i + 1) * D], rot2[:, 0:D], AF.Identity)

        score_sb = const.tile([P, NEG], f32)

        # ---------- main loop: NSTEP single-row gathers ----------
        for s in range(NSTEP):
            tj = hoisted[s] if s < HOIST else emit_gather(s)
            if PAIRSUM_VARIANT == "G2":
                sqc = sqp.tile([P, G * D2], f32, tag="sqc")
                nc.vector._custom_dve(pairsum, out=sqc[:], in0=tj[:], in1=rot2[:])
                acts = [sqc[:, i * D2 : (i + 1) * D2] for i in range(G)]
            else:
                sqc = sqp.tile([P, G * D], f32, tag="sqc")
                nc.vector._custom_dve(pairsum, out=sqc[:], in0=tj[:], in1=rot2[:])
                acts = [sqc[:, i * D + 1 : (i + 1) * D : 2] for i in range(G)]
            for i in range(G):
                srt = psc.tile([P, D2], f32, tag="srt")
                nc.scalar.activation(
                    srt[:], acts[i], AF.Sqrt,
                    accum_out=score_sb[:, s * G + i : s * G + i + 1],
                )

        # ---------- finale: score = GAMMA - colsum ----------
        out_t = const.tile([P, NEG], f32)
        nc.scalar.activation(out_t[:], score_sb[:], AF.Identity, scale=-1.0, bias=b_gamma)
        nc.sync.dma_start(out=score[:], in_=out_t[:])

    nc.compile()
    return nc


def _get_nc():
    global _CACHED_NC
    if _CACHED_NC is None:
        _CACHED_NC = _build_nc()
    return _CACHED_NC


def _run(inputs, **spmd_kwargs):
    hp = np.ascontiguousarray(np.asarray(inputs["head_part"], dtype=np.int32))
    tp = np.asarray(inputs["tail_part"], dtype=np.int32)
    rel = np.ascontiguousarray(np.asarray(inputs["relation_embedding"], dtype=np.float32))
    ent = np.asarray(inputs["entity_embedding"], dtype=np.float32)

    # interleave entity columns: ent_il[:, 2d] = ent[:, d], ent_il[:, 2d+1] = ent[:, 500+d]
    ent_il = np.ascontiguousarray(
        ent.reshape(N_ENT, 2, D2).transpose(0, 2, 1).reshape(N_ENT, D)
    )
    # sort each batch row's neg indices for HBM locality; unpermute after
    order = np.argsort(tp, axis=1).astype(np.int32)
    tp_sorted = np.ascontiguousarray(np.take_along_axis(tp, order, axis=1))

    in_maps = []
    for c in range(N_CORES):
        sl = slice(c * B_LOC, (c + 1) * B_LOC)
        in_maps.append(
            {
                "head_part": hp[sl],
                "tail_part": tp_sorted[sl],
                "relation_embedding": rel,
                "entity_embedding": ent_il,
            }
        )
    res = run_bass_kernel_spmd(_get_nc(), in_maps, core_ids=list(range(N_CORES)), **spmd_kwargs)
    out_sorted = np.concatenate([r["score"] for r in res.results], axis=0)
    out = np.empty_like(out_sorted)
    np.put_along_axis(out, order, out_sorted, axis=1)
    return out, res


def kernel(**inputs) -> np.ndarray:
    return _run(inputs)[0]


def kernel_traced(**inputs):
    """Like kernel() but returns (output, BassKernelResults) with HW profile."""
    return _run(inputs, trace=True)
